# revision 1
# baseline (speedup 1.0000x reference)
"""TRN2 Bass kernel for nn_Block_27994596835704 (GNN message passing block).

Strategy (8 NeuronCores, SPMD):
  - Shard destination nodes: core c owns rows [c*6250, (c+1)*6250) of both
    t_out and x_out outputs.
  - segment_sum: edges dst-sorted into 128-edge tiles grouped by 16-dst
    windows. Source rows are gathered from HBM f16 tables via indirect DMA
    (one descriptor per row) into [128 edges, 128 feat] SBUF tiles; a
    one-hot S matrix (built on-chip from CPU-provided window-local dst ids)
    is the stationary operand of a PE matmul accumulating each window's
    partial sums in PSUM (f32).
  - SPMD regularity: all cores share one instruction stream; per-window
    tile counts are the max over cores (short cores pad with dummy slots:
    src=0, dstid=-1 -> zero one-hot column -> no contribution).
  - MLPs/LayerNorm run in transposed [feat, rows] layout with fp32r
    matmuls; LN statistics via fp32 ones-matmuls across partitions.
"""
import numpy as np
from contextlib import ExitStack

import concourse.bass as bass
import concourse.bacc as bacc
import concourse.tile as tile
from concourse import mybir
from concourse.bass_utils import run_bass_kernel_spmd

F32 = mybir.dt.float32
F32R = mybir.dt.float32r
F16 = mybir.dt.float16
I16 = mybir.dt.int16
I32 = mybir.dt.int32
I64 = mybir.dt.int64

N = 50000
D = 128
NC = 8
RPC = N // NC          # 6250 dst rows per core
W = 128                # dsts per psum window
NWIN = (RPC + W - 1) // W
CHUNK = 32             # gather tiles per indirect-DMA call
GATHER_MODE = "single"    # "i64": [128,CHUNK] int64 offsets; "single": [128,1] per tile
LN_EPS = 1e-5
_cache = {}


def _prep_set(e, lo, hi):
    mask = (e[1] >= lo) & (e[1] < hi)
    src = np.ascontiguousarray(e[0][mask]).astype(np.int64)
    dstl = np.ascontiguousarray(e[1][mask]).astype(np.int64) - lo
    order = np.argsort(dstl, kind="stable")
    src, dstl = src[order], dstl[order]
    win = dstl // W
    bounds = np.searchsorted(win, np.arange(NWIN + 1))
    return [(src[bounds[w]:bounds[w + 1]],
             (dstl[bounds[w]:bounds[w + 1]] - w * W).astype(np.int16))
            for w in range(NWIN)]


def _schedule(per_core_sets):
    tpw = np.zeros(NWIN, dtype=np.int64)
    for wins in per_core_sets:
        for w, (s, _) in enumerate(wins):
            tpw[w] = max(tpw[w], (len(s) + 127) // 128)
    return np.maximum(tpw, 1)


def _padded_ntiles(tpw):
    nt = int(tpw.sum())
    return (nt + CHUNK - 1) // CHUNK * CHUNK


def _pack(wins, tpw):
    ntiles = int(tpw.sum())
    ntp = _padded_ntiles(tpw)
    gidx2 = np.zeros((128, ntp), dtype=np.int32)
    dstid = np.full((128, ntp), -1, dtype=np.int16)
    t0 = 0
    for w in range(NWIN):
        s, dl = wins[w]
        nt = int(tpw[w])
        n = len(s)
        buf_s = np.zeros(nt * 128, dtype=np.int32)
        buf_d = np.full(nt * 128, -1, dtype=np.int16)
        buf_s[:n] = s
        buf_d[:n] = dl
        gidx2[:, t0:t0 + nt] = buf_s.reshape(nt, 128).T
        dstid[:, t0:t0 + nt] = buf_d.reshape(nt, 128).T
        t0 += nt
    if GATHER_MODE == "i64":
        pairs = np.zeros((128, 2 * ntp), dtype=np.int32)
        pairs[:, 0::2] = gidx2      # little-endian lo word of int64
        return np.ascontiguousarray(pairs), dstid
    return np.ascontiguousarray(gidx2), dstid


def _build_program(tpwA, tpwB):
    ntA, ntB = _padded_ntiles(tpwA), _padded_ntiles(tpwB)
    nc = bacc.Bacc("TRN2", target_bir_lowering=False, debug=False, num_devices=NC)
    d = {}

    def din(name, shape, dt):
        d[name] = nc.dram_tensor(name, shape, dt, kind="ExternalInput").ap()

    def dout(name, shape, dt):
        d[name] = nc.dram_tensor(name, shape, dt, kind="ExternalOutput").ap()

    din("t16", [N, D], F16)
    din("x16", [N, D], F16)
    gw = 2 if GATHER_MODE == "i64" else 1
    din("gidxA", [128, gw * ntA], I32)
    din("gidxB", [128, gw * ntB], I32)
    din("dstidA", [128, ntA], I16)
    din("dstidB", [128, ntB], I16)
    din("iotaw", [128, W], I16)
    din("tT", [D, RPC], F32)
    din("xT", [D, RPC], F32)
    for nm in ["W1a", "W2a", "W1b", "W2b", "Wo", "Wf1", "Wf2"]:
        din(nm, [D, D], F32)
    # bias columns: 0:b1a 1:b2a+b2b 2:b1b 3:bo 4:bf1 5:bf2 6:ln_g 7:ln_b
    din("biases", [D, 8], F32)
    dout("toutT", [D, RPC], F32)
    dout("xoutT", [D, RPC], F32)

    with tile.TileContext(nc) as tc, ExitStack() as ctx:
        pool = ctx.enter_context(tc.tile_pool(name="sbuf", bufs=1))
        gpool = ctx.enter_context(tc.tile_pool(name="g", bufs=3))
        spool = ctx.enter_context(tc.tile_pool(name="s", bufs=3))
        iopool = ctx.enter_context(tc.tile_pool(name="io", bufs=2))
        mpool = ctx.enter_context(tc.tile_pool(name="m", bufs=1))
        psA = ctx.enter_context(tc.tile_pool(name="psA", bufs=2, space="PSUM"))
        psM = ctx.enter_context(tc.tile_pool(name="psM", bufs=2, space="PSUM"))
        psT = ctx.enter_context(tc.tile_pool(name="psT", bufs=2, space="PSUM"))
        psL = ctx.enter_context(tc.tile_pool(name="psL", bufs=2, space="PSUM"))

        iota_t = pool.tile([128, W], I16)
        nc.sync.dma_start(out=iota_t[:], in_=d["iotaw"][:])
        wt = {}
        for nm in ["W1a", "W2a", "W1b", "W2b", "Wo", "Wf1", "Wf2"]:
            w_f = pool.tile([D, D], F32, tag=f"w_{nm}")
            nc.sync.dma_start(out=w_f[:], in_=d[nm][:])
            w_r = pool.tile([D, D], F32R, tag=f"wr_{nm}")
            nc.vector.tensor_copy(w_r[:], w_f[:])
            wt[nm] = w_r
        bias_t = pool.tile([D, 8], F32)
        nc.sync.dma_start(out=bias_t[:], in_=d["biases"][:])
        ones_f32 = pool.tile([128, 1], F32)
        nc.vector.memset(ones_f32[:], 1.0)
        eps_t = pool.tile([1, 1], F32)
        nc.vector.memset(eps_t[:], LN_EPS)
        ones_r = pool.tile([1, 128], F32)
        nc.vector.memset(ones_r[:], 1.0)
        from concourse.masks import make_identity
        ident = pool.tile([128, 128], F32)
        make_identity(nc, ident[:])

        def aggregate(set_name, tpw, ntiles, tbl_ap, gidx_ap, dstid_ap):
            gw = 2 if GATHER_MODE == "i64" else 1
            gidx_t = pool.tile([128, gw * ntiles], I32, tag=f"gidx{set_name}")
            nc.sync.dma_start(out=gidx_t[:], in_=gidx_ap[:])
            dstid_t = pool.tile([128, ntiles], I16, tag=f"dstid{set_name}")
            nc.sync.dma_start(out=dstid_t[:], in_=dstid_ap[:])

            agg_tiles = []
            win_start = np.zeros(NWIN, dtype=np.int64)
            win_start[1:] = np.cumsum(tpw)[:-1]
            nchunks = (ntiles + CHUNK - 1) // CHUNK
            G = [None] * nchunks
            S = [None] * nchunks

            def ensure_chunk(ci):
                if G[ci] is not None:
                    return
                c0 = ci * CHUNK
                g = gpool.tile([128, CHUNK, 128], F16, tag="G")
                if GATHER_MODE == "i64":
                    nc.gpsimd.indirect_dma_start(
                        out=g[:],
                        out_offset=None,
                        in_=tbl_ap[:],
                        in_offset=bass.IndirectOffsetOnAxis(
                            ap=gidx_t[:, 2 * c0:2 * (c0 + CHUNK)].bitcast(I64), axis=0),
                    )
                else:
                    for c in range(CHUNK):
                        nc.gpsimd.indirect_dma_start(
                            out=g[:, c, :],
                            out_offset=None,
                            in_=tbl_ap[:],
                            in_offset=bass.IndirectOffsetOnAxis(
                                ap=gidx_t[:, c0 + c:c0 + c + 1], axis=0),
                        )
                s = spool.tile([128, CHUNK, W], F16, tag="S")
                nc.vector.tensor_tensor(
                    out=s[:],
                    in0=dstid_t[:, c0:c0 + CHUNK, None].broadcast_to((128, CHUNK, W)),
                    in1=iota_t[:, None, :].broadcast_to((128, CHUNK, W)),
                    op=mybir.AluOpType.is_equal,
                )
                G[ci], S[ci] = g, s

            for w in range(NWIN):
                cur_ps = psA.tile([128, 128], F32, tag="aggps")
                nt = int(tpw[w])
                t0 = int(win_start[w])
                for k in range(nt):
                    ti = t0 + k
                    ci, cj = divmod(ti, CHUNK)
                    ensure_chunk(ci)
                    nc.tensor.matmul(
                        out=cur_ps[:],
                        lhsT=S[ci][:, cj, :],
                        rhs=G[ci][:, cj, :],
                        start=(k == 0), stop=(k == nt - 1),
                    )
                a = pool.tile([128, 128], F32, tag=f"agg{set_name}_{w}")
                nc.vector.tensor_copy(a[:], cur_ps[:])
                agg_tiles.append(a)
            return agg_tiles

        aggA = aggregate("A", tpwA, ntA, d["t16"], d["gidxA"], d["dstidA"])
        aggB = aggregate("B", tpwB, ntB, d["x16"], d["gidxB"], d["dstidB"])

        REL = mybir.ActivationFunctionType.Relu
        SQ = mybir.ActivationFunctionType.Square
        SQRT = mybir.ActivationFunctionType.Sqrt

        nsup = (RPC + 511) // 512
        for si in range(nsup):
            c0 = si * 512
            cw = min(512, RPC - c0)
            tTs = iopool.tile([128, 512], F32, tag="tTs")
            nc.sync.dma_start(out=tTs[:, :cw], in_=d["tT"][:, c0:c0 + cw])
            xTs = iopool.tile([128, 512], F32, tag="xTs")
            nc.sync.dma_start(out=xTs[:, :cw], in_=d["xT"][:, c0:c0 + cw])

            h0a = mpool.tile([128, 512], F32R, tag="h0a")
            h0b = mpool.tile([128, 512], F32R, tag="h0b")
            for (h0, aggt) in ((h0a, aggA), (h0b, aggB)):
                for k in range((cw + 127) // 128):
                    r0 = c0 + k * 128
                    rw = min(128, RPC - r0)
                    pt = psT.tile([128, 128], F32, tag="tp")
                    nc.tensor.transpose(
                        out=pt[:], in_=aggt[r0 // 128][:], identity=ident[:])
                    nc.vector.tensor_add(
                        h0[:, k * 128:k * 128 + rw],
                        pt[:, :rw], tTs[:, k * 128:k * 128 + rw])

            def mm(lhsT, rhs, n=cw):
                p = psM.tile([128, 512], F32, tag="mlp")
                nc.tensor.matmul(out=p[:, :n], lhsT=lhsT[:], rhs=rhs,
                                 start=True, stop=True)
                return p

            def gin(h0, w1, w2, b1_col):
                u = mm(wt[w1], h0[:, :cw])
                ur = mpool.tile([128, 512], F32R, tag="ur")
                nc.scalar.activation(ur[:, :cw], u[:, :cw], REL,
                                     bias=bias_t[:, b1_col:b1_col + 1], scale=1.0)
                return mm(wt[w2], ur[:, :cw])

            ha = gin(h0a, "W1a", "W2a", 0)
            s1 = mpool.tile([128, 512], F32, tag="s1")
            nc.vector.tensor_add(s1[:, :cw], tTs[:, :cw], ha[:, :cw])
            hb = gin(h0b, "W1b", "W2b", 2)
            nc.vector.tensor_add(s1[:, :cw], s1[:, :cw], hb[:, :cw])
            t2 = mpool.tile([128, 512], F32R, tag="t2")
            nc.scalar.activation(t2[:, :cw], s1[:, :cw], REL,
                                 bias=bias_t[:, 1:2], scale=1.0)
            o_ps = mm(wt["Wo"], t2[:, :cw])
            o1r = mpool.tile([128, 512], F32, tag="o1r")
            nc.scalar.activation(o1r[:, :cw], o_ps[:, :cw], REL,
                                 bias=bias_t[:, 3:4], scale=1.0)
            sq = mpool.tile([128, 512], F32, tag="sq")
            nc.scalar.activation(sq[:, :cw], o1r[:, :cw], SQ)
            cs1 = psL.tile([1, 512], F32, tag="ln1")
            nc.tensor.matmul(out=cs1[:, :cw], lhsT=ones_f32[:], rhs=o1r[:, :cw],
                             start=True, stop=True)
            cs2 = psL.tile([1, 512], F32, tag="ln1")
            nc.tensor.matmul(out=cs2[:, :cw], lhsT=ones_f32[:], rhs=sq[:, :cw],
                             start=True, stop=True)
            mean = mpool.tile([1, 512], F32, tag="mean")
            nc.vector.tensor_scalar_mul(mean[:, :cw], cs1[:, :cw], 1.0 / 128.0)
            ex2 = mpool.tile([1, 512], F32, tag="ex2")
            nc.vector.tensor_scalar_mul(ex2[:, :cw], cs2[:, :cw], 1.0 / 128.0)
            m2 = mpool.tile([1, 512], F32, tag="m2")
            nc.vector.tensor_mul(m2[:, :cw], mean[:, :cw], mean[:, :cw])
            var = mpool.tile([1, 512], F32, tag="var")
            nc.vector.tensor_sub(var[:, :cw], ex2[:, :cw], m2[:, :cw])
            sd = mpool.tile([1, 512], F32, tag="sd")
            nc.scalar.activation(sd[:, :cw], var[:, :cw], SQRT,
                                 bias=eps_t[:], scale=1.0)
            rstd = mpool.tile([1, 512], F32, tag="rstd")
            nc.vector.reciprocal(rstd[:, :cw], sd[:, :cw])
            mb = mm(ones_r, mean[:, :cw])
            ycen = mpool.tile([128, 512], F32, tag="ycen")
            nc.vector.tensor_sub(ycen[:, :cw], o1r[:, :cw], mb[:, :cw])
            rb = mm(ones_r, rstd[:, :cw])
            y = mpool.tile([128, 512], F32, tag="y")
            nc.vector.tensor_mul(y[:, :cw], ycen[:, :cw], rb[:, :cw])
            ygb = mpool.tile([128, 512], F32, tag="ygb")
            nc.vector.tensor_scalar(ygb[:, :cw], y[:, :cw],
                                    bias_t[:, 6:7], bias_t[:, 7:8],
                                    mybir.AluOpType.mult, mybir.AluOpType.add)
            touts = iopool.tile([128, 512], F32, tag="touts")
            nc.vector.tensor_add(touts[:, :cw], t2[:, :cw], ygb[:, :cw])
            nc.sync.dma_start(out=d["toutT"][:, c0:c0 + cw], in_=touts[:, :cw])

            xr = mpool.tile([128, 512], F32R, tag="xr")
            nc.vector.tensor_copy(xr[:, :cw], xTs[:, :cw])
            f1 = mm(wt["Wf1"], xr[:, :cw])
            f1r = mpool.tile([128, 512], F32R, tag="f1r")
            nc.scalar.activation(f1r[:, :cw], f1[:, :cw], REL,
                                 bias=bias_t[:, 4:5], scale=1.0)
            f2 = mm(wt["Wf2"], f1r[:, :cw])
            xo = mpool.tile([128, 512], F32, tag="xo")
            nc.vector.tensor_add(xo[:, :cw], xTs[:, :cw], f2[:, :cw])
            xouts = iopool.tile([128, 512], F32, tag="xouts")
            nc.vector.tensor_scalar(xouts[:, :cw], xo[:, :cw],
                                    ones_f32[:], bias_t[:, 5:6],
                                    mybir.AluOpType.mult, mybir.AluOpType.add)
            nc.sync.dma_start(out=d["xoutT"][:, c0:c0 + cw], in_=xouts[:, :cw])

    nc.compile()
    return nc


def kernel(x, t, e_t, e_xct, W1a, b1a, W2a, b2a, W1b, b1b, W2b, b2b,
           Wo, bo, ln_g, ln_b, Wf1, bf1, Wf2, bf2):
    x = np.asarray(x, dtype=np.float32)
    t = np.asarray(t, dtype=np.float32)
    e_t = np.asarray(e_t)
    e_xct = np.asarray(e_xct)

    t16 = t.astype(np.float16)
    x16 = x.astype(np.float16)

    setsA = [_prep_set(e_t, c * RPC, (c + 1) * RPC) for c in range(NC)]
    setsB = [_prep_set(e_xct, c * RPC, (c + 1) * RPC) for c in range(NC)]
    tpwA = _schedule(setsA)
    tpwB = _schedule(setsB)

    key = (tuple(tpwA.tolist()), tuple(tpwB.tolist()))
    if key not in _cache:
        _cache[key] = _build_program(tpwA, tpwB)
    nc = _cache[key]

    iotaw = np.tile(np.arange(W, dtype=np.int16), (128, 1))
    b2ab = np.asarray(b2a, np.float32) + np.asarray(b2b, np.float32)
    biases = np.stack([np.asarray(v, np.float32) for v in
                       [b1a, b2ab, b1b, bo, bf1, bf2, ln_g, ln_b]], axis=1)
    shared = {
        "t16": t16, "x16": x16, "iotaw": iotaw, "biases": biases,
        "W1a": np.asarray(W1a, np.float32), "W2a": np.asarray(W2a, np.float32),
        "W1b": np.asarray(W1b, np.float32), "W2b": np.asarray(W2b, np.float32),
        "Wo": np.asarray(Wo, np.float32),
        "Wf1": np.asarray(Wf1, np.float32), "Wf2": np.asarray(Wf2, np.float32),
    }
    in_maps = []
    for c in range(NC):
        gA, dA = _pack(setsA[c], tpwA)
        gB, dB = _pack(setsB[c], tpwB)
        lo, hi = c * RPC, (c + 1) * RPC
        in_maps.append({
            **shared,
            "gidxA": gA, "dstidA": dA, "gidxB": gB, "dstidB": dB,
            "tT": np.ascontiguousarray(t[lo:hi].T),
            "xT": np.ascontiguousarray(x[lo:hi].T),
        })

    res = run_bass_kernel_spmd(nc, in_maps, list(range(NC)))
    t_out = np.concatenate([res.results[c]["toutT"].T for c in range(NC)], axis=0)
    x_out = np.concatenate([res.results[c]["xoutT"].T for c in range(NC)], axis=0)
    return (x_out.astype(np.float32), t_out.astype(np.float32))

